# revision 14
# baseline (speedup 1.0000x reference)
"""Trainium2 Bass kernel for BasicCNN+LSTM (conv3x3+ReLU+GAP -> custom LSTM scan).

Self-contained: hardcodes shapes/sharding. Data-parallel over batch B=8 across
8 NeuronCores; each core processes one batch element end-to-end, the host
gathers the 8 [1,32] results.

Per-core device pipeline (per frame t of 24):
  - DMA a host-prepacked, channel-deinterleaved "stack" [36, 56*112] bf16 into
    an SBUF row-band (partition p = c*12 + dx*4 + r holds plane c shifted by
    (dx-1, parity row r)); 2 round-robin bands at partitions {0, 64} -> 2
    concurrent PE row-group streams.
  - Conv as ONE K=36 matmul per PSUM tile (contraction over the full 3x3x3
    receptive field of a vertically-packed pixel pair; M=96 = 2 px x 48
    filters, N=448 = 4 ja-blocks x 112 w, contiguous rhs). 14 tiles/frame.
  - Fused ReLU(+conv-bias)+GAP: ScalarE activation(Relu, bias, accum_out) and
    VectorE tensor_scalar((x+bias) max 0, accum_out), 7/7 split with separate
    per-engine gsum tiles (avoids cross-engine WAW serialization).
  - Tiny LSTM-ish scan step on-chip ([1,96] gates in free layout; the
    reference's state-order swap bug is reproduced faithfully). Scan step t
    is emitted after frame t+2's conv to avoid FIFO head-of-line blocking.
"""
import sys
if '/opt/trn_rl_repo' not in sys.path:
    sys.path.insert(0, '/opt/trn_rl_repo')

import numpy as np
import ml_dtypes

import concourse.bass as bass
import concourse.mybir as mybir
import concourse.tile as tile
from concourse.vector_clock import ScopedClock
from concourse.bass_utils import run_bass_kernel_spmd

# ---------------------------------------------------------------- constants
B, T, H, W, C, F, U = 8, 24, 112, 112, 3, 48, 32
JA = 56            # vertical pixel-pair blocks (112 rows / 2)
KP = 36            # stack partitions: 3 c x 3 dx x 4 window rows
M = 96             # 2 pixels x 48 filters
NSB = 14           # superblocks (PSUM tiles) per frame
NQ = 448           # columns per superblock = 4 ja-blocks x 112 w
FREE = JA * W      # stack free size per partition (elements)

FP32 = mybir.dt.float32
BF16 = mybir.dt.bfloat16

LAST_RESULTS = None  # BassKernelResults of the most recent run (for test.py)

# ------------------------------------------------- TileContext drain patch
# The container's walrus rejects >1 semaphore wait per instruction; Tile's
# kernel-tail drain aggregates all end-of-kernel waits onto one Drain.
# Spread them across single-wait NOPs on the sync engine instead.
def _patched_drain_and_barrier(self, tick_clock, wait_clock):
    nc = self.nc
    probe = nc.sync.nop(nofuse=True, hint="tail_waits")
    wait_clock.add_sem_waits(probe.ins, ScopedClock({None: tick_clock.global_clock}))
    waits = list(probe.ins.sync_info.on_wait or [])
    if len(waits) > 1:
        probe.ins.sync_info.on_wait = waits[:1]
        for i in range(1, len(waits)):
            extra = nc.sync.nop(nofuse=True, hint=f"tail_waits_{i}")
            si = extra.ins.sync_info
            if si is None:
                extra.ins.sync_info = mybir.SyncInfo(on_wait=[waits[i]], on_update=[])
            else:
                si.on_wait = [waits[i]]
    nc.sync.drain()
    nc.all_engine_barrier()
    popped = nc._tile_sem_poison_stack.pop()
    assert popped is self._sem_poison
    nc.clear_and_free_semaphores(list(self.sems.allocated().values()))
    nc.all_engine_barrier()


tile.TileContext._drain_and_barrier = _patched_drain_and_barrier

# Same walrus restriction for regular instructions: spill extra sem waits
# onto preceding same-engine NOPs at commit time.
_orig_commit = tile.TileContext._commit_instruction


def _patched_commit(self, inst, *args, **kwargs):
    si = getattr(inst, 'sync_info', None)
    if si is not None and si.on_wait and len(si.on_wait) > 1 \
            and inst.engine != mybir.EngineType.Unassigned:
        waits = list(si.on_wait)
        si.on_wait = waits[-1:]
        for w in waits[:-1]:
            nop = mybir.InstNoOp(
                name=self.nc.get_next_instruction_name(),
                ins=[], outs=[], bass_is_fusable=False)
            nop.engine = inst.engine
            nop.sync_info = mybir.SyncInfo(on_wait=[w], on_update=[])
            _orig_commit(self, nop, *args, **kwargs)
    return _orig_commit(self, inst, *args, **kwargs)


tile.TileContext._commit_instruction = _patched_commit


# ------------------------------------------------------------- device code
def _build_bass(use_gbias=True):
    _build_bass.use_gbias = use_gbias
    nc = bass.Bass('TRN2', target_bir_lowering=False, debug=False)

    xin = nc.dram_tensor('xin', [T, KP, FREE], BF16, kind='ExternalInput')
    smat_d = nc.dram_tensor('smat', [KP, M], BF16, kind='ExternalInput')
    cbias_d = nc.dram_tensor('cbias', [M, 1], FP32, kind='ExternalInput')
    wfeat_d = nc.dram_tensor('wfeat', [M, 96], FP32, kind='ExternalInput')
    whid_d = nc.dram_tensor('whid', [U, 96], FP32, kind='ExternalInput')
    gbias_d = nc.dram_tensor('gbias', [1, 96], FP32, kind='ExternalInput')
    outh_d = nc.dram_tensor('outh', [1, U], FP32, kind='ExternalOutput')

    Relu = mybir.ActivationFunctionType.Relu
    Sigmoid = mybir.ActivationFunctionType.Sigmoid
    Tanh = mybir.ActivationFunctionType.Tanh
    Amax = mybir.AluOpType.max
    Aadd = mybir.AluOpType.add

    with tile.TileContext(nc) as tc:
        const = tc.alloc_tile_pool(name='const', bufs=1)
        state = tc.alloc_tile_pool(name='state', bufs=1)
        stackp = tc.alloc_tile_pool(name='stack', bufs=2)
        psum = tc.alloc_tile_pool(name='psum', bufs=7, space='PSUM')
        spsum = tc.alloc_tile_pool(name='spsum', bufs=1, space='PSUM')
        gs = tc.alloc_tile_pool(name='gs', bufs=6)
        fs = tc.alloc_tile_pool(name='fs', bufs=6)
        ga_pool = tc.alloc_tile_pool(name='ga', bufs=4)
        tmp = tc.alloc_tile_pool(name='tmp', bufs=6)

        # constants
        sc_all = const.tile([128, M], BF16, tag='sc')
        for s in range(2):
            nc.sync.dma_start(sc_all[64 * s:64 * s + KP, :], smat_d[:])
        cbias = const.tile([M, 1], FP32, tag='cb')
        nc.sync.dma_start(cbias[:], cbias_d[:])
        wfeat = const.tile([M, 96], FP32, tag='wf')
        nc.sync.dma_start(wfeat[:], wfeat_d[:])
        whid = const.tile([U, 96], FP32, tag='wh')
        nc.sync.dma_start(whid[:], whid_d[:])
        gbias = const.tile([1, 96], FP32, tag='gb')
        nc.sync.dma_start(gbias[:], gbias_d[:])
        ident = const.tile([1, 1], FP32, tag='id')
        nc.vector.memset(ident[:], 1.0)
        zeros448 = const.tile([M, 512], FP32, tag='z448')
        nc.vector.memset(zeros448[:], 0.0)

        # persistent scan state
        cellv = state.tile([1, U], FP32, tag='cell')    # prev new_cell
        hidv = state.tile([1, U], FP32, tag='hid')      # prev new_hidden
        cell_part = state.tile([U, 1], FP32, tag='cp')  # new_cell, transposed
        nc.vector.memset(cellv[:], 0.0)
        nc.vector.memset(hidv[:], 0.0)
        nc.vector.memset(cell_part[:], 0.0)

        fsums = [None] * T

        # N=512 flat windows over the frame's 6272 columns: 12 full + 1 tail
        # of 128. ACT takes 7 (even idx), DVE takes 6 (odd idx, incl. tail).
        WINS = [(i * 512, min(512, FREE - i * 512)) for i in range((FREE + 511) // 512)]
        ACT_WIN = [i for i in range(len(WINS)) if i % 2 == 0]

        def emit_conv(t):
            s = t % 2
            if s == 0:
                emit_conv.round_tile = stackp.tile([128, FREE], BF16, tag='stk')
            band = emit_conv.round_tile[64 * s:64 * s + KP, :]
            nc.sync.dma_start(band, xin[t])
            lhsT = sc_all[64 * s:64 * s + KP, :]

            na = len(ACT_WIN)
            nb = len(WINS) - na
            gsumA = gs.tile([M, na], FP32, tag='gsumA')
            gsumB = gs.tile([M, nb], FP32, tag='gsumB')
            ia = ib = 0
            for q, (off, n) in enumerate(WINS):
                ps = psum.tile([M, 512], FP32, tag='ps')
                nc.tensor.matmul(ps[:, 0:n], lhsT, band[:, off:off + n],
                                 start=True, stop=True,
                                 tile_position=(64 * s, 0))
                if q in ACT_WIN:
                    nc.scalar.activation(ps[:, 0:n], ps[:, 0:n], Relu,
                                         bias=cbias[:],
                                         accum_out=gsumA[:, ia:ia + 1])
                    ia += 1
                else:
                    nc.vector.scalar_tensor_tensor(
                        out=ps[:, 0:n], in0=ps[:, 0:n], scalar=cbias[:],
                        in1=zeros448[:, 0:n], op0=Aadd, op1=Amax,
                        accum_out=gsumB[:, ib:ib + 1])
                    ib += 1

            fsA = tmp.tile([M, 1], FP32, tag='fsA')
            nc.vector.reduce_sum(fsA[:], gsumA[:], axis=mybir.AxisListType.X)
            fsB = tmp.tile([M, 1], FP32, tag='fsB')
            nc.vector.reduce_sum(fsB[:], gsumB[:], axis=mybir.AxisListType.X)
            fsum = fs.tile([M, 1], FP32, tag='fsum')
            nc.vector.tensor_add(fsum[:], fsA[:], fsB[:])
            fsums[t] = fsum

        def emit_scan(t):
            # z-hidden part = prev new_cell (reference's state-order swap bug)
            fsum = fsums[t]
            pg = spsum.tile([1, 96], FP32, tag='sps')
            nc.tensor.matmul(pg[:], fsum[:], wfeat[:], start=True, stop=False)
            nc.tensor.matmul(pg[:], cell_part[:], whid[:], start=False, stop=True)
            if _build_bass.use_gbias:
                gpre = ga_pool.tile([1, 96], FP32, tag='gpre')
                nc.vector.tensor_add(gpre[:], pg[:], gbias[:])
            else:
                gpre = pg
            ga = ga_pool.tile([1, 96], FP32, tag='ga')
            nc.scalar.activation(ga[:, 0:2 * U], gpre[:, 0:2 * U], Sigmoid)
            nc.scalar.activation(ga[:, 2 * U:3 * U], gpre[:, 2 * U:3 * U], Tanh)
            t1 = tmp.tile([1, U], FP32, tag='t1')
            nc.vector.tensor_mul(t1[:], ga[:, 0:U], hidv[:])       # sig1*prev_hid
            t2 = tmp.tile([1, U], FP32, tag='t2')
            nc.vector.tensor_mul(t2[:], ga[:, U:2 * U], ga[:, 2 * U:3 * U])
            nc.vector.tensor_add(cellv[:], t1[:], t2[:])           # new_cell
            t3 = tmp.tile([1, U], FP32, tag='t3')
            nc.scalar.activation(t3[:], cellv[:], Tanh)
            nc.vector.tensor_mul(hidv[:], cellv[:], t3[:])         # new_hidden
            if t < T - 1:
                ph = spsum.tile([U, 1], FP32, tag='sps')
                nc.tensor.transpose(ph[:], cellv[:], ident[:])
                nc.vector.tensor_copy(cell_part[:], ph[:])

        for t in range(T):
            emit_conv(t)
            if t >= 2:
                emit_scan(t - 2)
        emit_scan(T - 2)
        emit_scan(T - 1)

        nc.sync.dma_start(outh_d[:], hidv[:])

        for p in (tmp, ga_pool, fs, gs, spsum, psum, stackp, state, const):
            p.release()

    return nc


# -------------------------------------------------------------- host prep
def _prep_inputs(x, conv_w, conv_b, W1, b1, W2, b2, W3, b3):
    x = np.asarray(x, np.float32)
    conv_w = np.asarray(conv_w, np.float32)
    conv_b = np.asarray(conv_b, np.float32)

    xp = np.zeros((B, T, H + 2, W + 2, C), np.float32)
    xp[:, :, 1:H + 1, 1:W + 1, :] = x
    xin2 = np.empty((B, T, KP, JA, W), np.float32)
    rows = 2 * np.arange(JA)
    for c in range(3):
        for dx in range(3):
            for r in range(4):
                p = c * 12 + dx * 4 + r
                xin2[:, :, p] = np.moveaxis(
                    xp[:, :, rows + r, dx:dx + W, c], 0, 2)
    xin2 = xin2.reshape(B, T, KP, FREE).astype(ml_dtypes.bfloat16)

    smat = np.zeros((KP, M), np.float32)
    for c in range(3):
        for dx in range(3):
            for r in range(4):
                p = c * 12 + dx * 4 + r
                for i in range(2):
                    dy = r - i
                    if 0 <= dy <= 2:
                        smat[p, i * F:(i + 1) * F] = conv_w[dy, dx, c, :]
    smat = smat.astype(ml_dtypes.bfloat16)
    cbias = np.concatenate([conv_b, conv_b]).reshape(M, 1).astype(np.float32)

    wfeat = np.zeros((M, 96), np.float32)
    whid = np.zeros((U, 96), np.float32)
    for g, Wg in enumerate([W1, W2, W3]):
        Wg = np.asarray(Wg, np.float32)
        for i in range(2):
            wfeat[i * F:(i + 1) * F, g * U:(g + 1) * U] = Wg[0:F, :] / float(H * W)
        whid[:, g * U:(g + 1) * U] = Wg[F:F + U, :]
    gbias = np.concatenate([np.asarray(b, np.float32) for b in (b1, b2, b3)])
    gbias = gbias.reshape(1, 96)

    return xin2, smat, cbias, wfeat, whid, gbias


# ------------------------------------------------------------------ kernel
def kernel(x, conv_w, conv_b, W1, b1, W2, b2, W3, b3, W4, b4):
    global LAST_RESULTS
    xin2, smat, cbias, wfeat, whid, gbias = _prep_inputs(
        x, conv_w, conv_b, W1, b1, W2, b2, W3, b3)

    nc = _build_bass(use_gbias=bool(np.any(gbias)))
    in_maps = [{
        'xin': np.ascontiguousarray(xin2[b]),
        'smat': smat,
        'cbias': cbias,
        'wfeat': wfeat,
        'whid': whid,
        'gbias': gbias,
    } for b in range(B)]

    res = run_bass_kernel_spmd(nc, in_maps, core_ids=list(range(B)))
    LAST_RESULTS = res
    out = np.stack([res.results[b]['outh'][0] for b in range(B)], axis=0)
    return out.astype(np.float32)


# revision 19
# speedup vs baseline: 1.0056x; 1.0056x over previous
"""Trainium2 Bass kernel for BasicCNN+LSTM (conv3x3+ReLU+GAP -> custom LSTM scan).

Self-contained: hardcodes shapes/sharding. Data-parallel over batch B=8 across
8 NeuronCores; each core processes one batch element end-to-end, the host
gathers the 8 [1,32] results.

Per-core device pipeline (per frame t of 24):
  - DMA a host-prepacked, channel-deinterleaved "stack" [36, 56*112] bf16 into
    an SBUF row-band (partition p = c*12 + dx*4 + r holds plane c shifted by
    (dx-1, parity row r)); 2 round-robin bands at partitions {0, 64} -> 2
    concurrent PE row-group streams.
  - Conv as ONE K=36 matmul per PSUM tile (contraction over the full 3x3x3
    receptive field of a vertically-packed pixel pair; M=96 = 2 px x 48
    filters, N=448 = 4 ja-blocks x 112 w, contiguous rhs). 14 tiles/frame.
  - Fused ReLU(+conv-bias)+GAP: ScalarE activation(Relu, bias, accum_out) and
    VectorE tensor_scalar((x+bias) max 0, accum_out), 7/7 split with separate
    per-engine gsum tiles (avoids cross-engine WAW serialization).
  - Tiny LSTM-ish scan step on-chip ([1,96] gates in free layout; the
    reference's state-order swap bug is reproduced faithfully). Scan step t
    is emitted after frame t+2's conv to avoid FIFO head-of-line blocking.
"""
import sys
if '/opt/trn_rl_repo' not in sys.path:
    sys.path.insert(0, '/opt/trn_rl_repo')

import numpy as np
import ml_dtypes

import concourse.bass as bass
import concourse.mybir as mybir
import concourse.tile as tile
from concourse.vector_clock import ScopedClock
from concourse.bass_utils import run_bass_kernel_spmd

# ---------------------------------------------------------------- constants
B, T, H, W, C, F, U = 8, 24, 112, 112, 3, 48, 32
JA = 56            # vertical pixel-pair blocks (112 rows / 2)
KP = 36            # stack partitions: 3 c x 3 dx x 4 window rows
M = 96             # 2 pixels x 48 filters
NSB = 14           # superblocks (PSUM tiles) per frame
NQ = 448           # columns per superblock = 4 ja-blocks x 112 w
FREE = JA * W      # stack free size per partition (elements)

FP32 = mybir.dt.float32
BF16 = mybir.dt.bfloat16

LAST_RESULTS = None  # BassKernelResults of the most recent run (for test.py)

# ------------------------------------------------- TileContext drain patch
# The container's walrus rejects >1 semaphore wait per instruction; Tile's
# kernel-tail drain aggregates all end-of-kernel waits onto one Drain.
# Spread them across single-wait NOPs on the sync engine instead.
def _patched_drain_and_barrier(self, tick_clock, wait_clock):
    nc = self.nc
    probe = nc.sync.nop(nofuse=True, hint="tail_waits")
    wait_clock.add_sem_waits(probe.ins, ScopedClock({None: tick_clock.global_clock}))
    waits = list(probe.ins.sync_info.on_wait or [])
    if len(waits) > 1:
        probe.ins.sync_info.on_wait = waits[:1]
        for i in range(1, len(waits)):
            extra = nc.sync.nop(nofuse=True, hint=f"tail_waits_{i}")
            si = extra.ins.sync_info
            if si is None:
                extra.ins.sync_info = mybir.SyncInfo(on_wait=[waits[i]], on_update=[])
            else:
                si.on_wait = [waits[i]]
    nc.sync.drain()
    nc.all_engine_barrier()
    popped = nc._tile_sem_poison_stack.pop()
    assert popped is self._sem_poison
    nc.clear_and_free_semaphores(list(self.sems.allocated().values()))
    nc.all_engine_barrier()


tile.TileContext._drain_and_barrier = _patched_drain_and_barrier

# Same walrus restriction for regular instructions: spill extra sem waits
# onto preceding same-engine NOPs at commit time.
_orig_commit = tile.TileContext._commit_instruction


def _patched_commit(self, inst, *args, **kwargs):
    si = getattr(inst, 'sync_info', None)
    if si is not None and si.on_wait and len(si.on_wait) > 1 \
            and inst.engine != mybir.EngineType.Unassigned:
        waits = list(si.on_wait)
        si.on_wait = waits[-1:]
        for w in waits[:-1]:
            nop = mybir.InstNoOp(
                name=self.nc.get_next_instruction_name(),
                ins=[], outs=[], bass_is_fusable=False)
            nop.engine = inst.engine
            nop.sync_info = mybir.SyncInfo(on_wait=[w], on_update=[])
            _orig_commit(self, nop, *args, **kwargs)
    return _orig_commit(self, inst, *args, **kwargs)


tile.TileContext._commit_instruction = _patched_commit

# NOTE: tried --enable-ldw-opt=true to dedupe the per-matmul stationary
# reloads (~70us of PE time); this walrus build fails in visitInstLdweights
# with it enabled, so the flag stays off.


# ------------------------------------------------------------- device code
def _build_bass(use_gbias=True):
    _build_bass.use_gbias = use_gbias
    nc = bass.Bass('TRN2', target_bir_lowering=False, debug=False)

    xin = nc.dram_tensor('xin', [T, KP, FREE], BF16, kind='ExternalInput')
    smat_d = nc.dram_tensor('smat', [KP, M], BF16, kind='ExternalInput')
    cbias_d = nc.dram_tensor('cbias', [M, 1], FP32, kind='ExternalInput')
    wfeat_d = nc.dram_tensor('wfeat', [M, 96], FP32, kind='ExternalInput')
    whid_d = nc.dram_tensor('whid', [U, 96], FP32, kind='ExternalInput')
    gbias_d = nc.dram_tensor('gbias', [1, 96], FP32, kind='ExternalInput')
    outh_d = nc.dram_tensor('outh', [1, U], FP32, kind='ExternalOutput')

    Relu = mybir.ActivationFunctionType.Relu
    Sigmoid = mybir.ActivationFunctionType.Sigmoid
    Tanh = mybir.ActivationFunctionType.Tanh
    Amax = mybir.AluOpType.max
    Aadd = mybir.AluOpType.add

    with tile.TileContext(nc) as tc:
        const = tc.alloc_tile_pool(name='const', bufs=1)
        state = tc.alloc_tile_pool(name='state', bufs=1)
        stackp = tc.alloc_tile_pool(name='stack', bufs=3)
        psum = tc.alloc_tile_pool(name='psum', bufs=7, space='PSUM')
        spsum = tc.alloc_tile_pool(name='spsum', bufs=1, space='PSUM')
        gs = tc.alloc_tile_pool(name='gs', bufs=6)
        fs = tc.alloc_tile_pool(name='fs', bufs=6)
        ga_pool = tc.alloc_tile_pool(name='ga', bufs=4)
        tmp = tc.alloc_tile_pool(name='tmp', bufs=6)

        # constants
        sc_all = const.tile([128, M], BF16, tag='sc')
        for s in range(2):
            nc.sync.dma_start(sc_all[64 * s:64 * s + KP, :], smat_d[:])
        cbias = const.tile([M, 1], FP32, tag='cb')
        nc.sync.dma_start(cbias[:], cbias_d[:])
        wfeat = const.tile([M, 96], FP32, tag='wf')
        nc.sync.dma_start(wfeat[:], wfeat_d[:])
        whid = const.tile([U, 96], FP32, tag='wh')
        nc.sync.dma_start(whid[:], whid_d[:])
        gbias = const.tile([1, 96], FP32, tag='gb')
        nc.sync.dma_start(gbias[:], gbias_d[:])
        ident = const.tile([1, 1], FP32, tag='id')
        nc.vector.memset(ident[:], 1.0)
        zeros448 = const.tile([M, 512], FP32, tag='z448')
        nc.vector.memset(zeros448[:], 0.0)

        # persistent scan state
        cellv = state.tile([1, U], FP32, tag='cell')    # prev new_cell
        hidv = state.tile([1, U], FP32, tag='hid')      # prev new_hidden
        cell_part = state.tile([U, 1], FP32, tag='cp')  # new_cell, transposed
        nc.vector.memset(cellv[:], 0.0)
        nc.vector.memset(hidv[:], 0.0)
        nc.vector.memset(cell_part[:], 0.0)

        fsums = [None] * T

        # N=512 flat windows over the frame's 6272 columns: 12 full + 1 tail
        # of 128. ACT takes 7 (even idx), DVE takes 6 (odd idx, incl. tail).
        WINS = [(i * 512, min(512, FREE - i * 512)) for i in range((FREE + 511) // 512)]
        ACT_WIN = [i for i in range(len(WINS)) if i % 2 == 0]

        rounds = [None] * (T // 2)

        def get_round(g):
            if rounds[g] is None:
                rt = stackp.tile([128, FREE], BF16, tag='stk')
                nc.sync.dma_start(rt[0:KP, :], xin[2 * g])
                nc.sync.dma_start(rt[64:64 + KP, :], xin[2 * g + 1])
                rounds[g] = rt
            return rounds[g]

        def emit_conv(t):
            s = t % 2
            rt = get_round(t // 2)
            if s == 0 and t // 2 + 1 < T // 2:
                get_round(t // 2 + 1)  # prefetch next round's DMAs
            band = rt[64 * s:64 * s + KP, :]
            lhsT = sc_all[64 * s:64 * s + KP, :]

            na = len(ACT_WIN)
            nb = len(WINS) - na
            gsumA = gs.tile([M, na], FP32, tag='gsumA')
            gsumB = gs.tile([M, nb], FP32, tag='gsumB')
            ia = ib = 0
            for q, (off, n) in enumerate(WINS):
                ps = psum.tile([M, 512], FP32, tag='ps')
                nc.tensor.matmul(ps[:, 0:n], lhsT, band[:, off:off + n],
                                 start=True, stop=True,
                                 tile_position=(64 * s, 0))
                if q in ACT_WIN:
                    nc.scalar.activation(ps[:, 0:n], ps[:, 0:n], Relu,
                                         bias=cbias[:],
                                         accum_out=gsumA[:, ia:ia + 1])
                    ia += 1
                else:
                    nc.vector.scalar_tensor_tensor(
                        out=ps[:, 0:n], in0=ps[:, 0:n], scalar=cbias[:],
                        in1=zeros448[:, 0:n], op0=Aadd, op1=Amax,
                        accum_out=gsumB[:, ib:ib + 1])
                    ib += 1

            fsA = tmp.tile([M, 1], FP32, tag='fsA')
            nc.vector.reduce_sum(fsA[:], gsumA[:], axis=mybir.AxisListType.X)
            fsB = tmp.tile([M, 1], FP32, tag='fsB')
            nc.vector.reduce_sum(fsB[:], gsumB[:], axis=mybir.AxisListType.X)
            fsum = fs.tile([M, 1], FP32, tag='fsum')
            nc.vector.tensor_add(fsum[:], fsA[:], fsB[:])
            fsums[t] = fsum

        def emit_scan(t):
            # z-hidden part = prev new_cell (reference's state-order swap bug)
            fsum = fsums[t]
            pg = spsum.tile([1, 96], FP32, tag='sps')
            nc.tensor.matmul(pg[:], fsum[:], wfeat[:], start=True, stop=False)
            nc.tensor.matmul(pg[:], cell_part[:], whid[:], start=False, stop=True)
            if _build_bass.use_gbias:
                gpre = ga_pool.tile([1, 96], FP32, tag='gpre')
                nc.vector.tensor_add(gpre[:], pg[:], gbias[:])
            else:
                gpre = pg
            ga = ga_pool.tile([1, 96], FP32, tag='ga')
            nc.scalar.activation(ga[:, 0:2 * U], gpre[:, 0:2 * U], Sigmoid)
            nc.scalar.activation(ga[:, 2 * U:3 * U], gpre[:, 2 * U:3 * U], Tanh)
            t1 = tmp.tile([1, U], FP32, tag='t1')
            nc.vector.tensor_mul(t1[:], ga[:, 0:U], hidv[:])       # sig1*prev_hid
            t2 = tmp.tile([1, U], FP32, tag='t2')
            nc.vector.tensor_mul(t2[:], ga[:, U:2 * U], ga[:, 2 * U:3 * U])
            nc.vector.tensor_add(cellv[:], t1[:], t2[:])           # new_cell
            t3 = tmp.tile([1, U], FP32, tag='t3')
            nc.scalar.activation(t3[:], cellv[:], Tanh)
            nc.vector.tensor_mul(hidv[:], cellv[:], t3[:])         # new_hidden
            if t < T - 1:
                ph = spsum.tile([U, 1], FP32, tag='sps')
                nc.tensor.transpose(ph[:], cellv[:], ident[:])
                nc.vector.tensor_copy(cell_part[:], ph[:])

        for t in range(T):
            if t >= 2:
                emit_scan(t - 2)
            emit_conv(t)
        emit_scan(T - 2)
        emit_scan(T - 1)

        nc.sync.dma_start(outh_d[:], hidv[:])

        for p in (tmp, ga_pool, fs, gs, spsum, psum, stackp, state, const):
            p.release()

    return nc


# -------------------------------------------------------------- host prep
def _prep_inputs(x, conv_w, conv_b, W1, b1, W2, b2, W3, b3):
    x = np.asarray(x, np.float32)
    conv_w = np.asarray(conv_w, np.float32)
    conv_b = np.asarray(conv_b, np.float32)

    xp = np.zeros((B, T, H + 2, W + 2, C), np.float32)
    xp[:, :, 1:H + 1, 1:W + 1, :] = x
    xin2 = np.empty((B, T, KP, JA, W), np.float32)
    rows = 2 * np.arange(JA)
    for c in range(3):
        for dx in range(3):
            for r in range(4):
                p = c * 12 + dx * 4 + r
                xin2[:, :, p] = np.moveaxis(
                    xp[:, :, rows + r, dx:dx + W, c], 0, 2)
    xin2 = xin2.reshape(B, T, KP, FREE).astype(ml_dtypes.bfloat16)

    smat = np.zeros((KP, M), np.float32)
    for c in range(3):
        for dx in range(3):
            for r in range(4):
                p = c * 12 + dx * 4 + r
                for i in range(2):
                    dy = r - i
                    if 0 <= dy <= 2:
                        smat[p, i * F:(i + 1) * F] = conv_w[dy, dx, c, :]
    smat = smat.astype(ml_dtypes.bfloat16)
    cbias = np.concatenate([conv_b, conv_b]).reshape(M, 1).astype(np.float32)

    wfeat = np.zeros((M, 96), np.float32)
    whid = np.zeros((U, 96), np.float32)
    for g, Wg in enumerate([W1, W2, W3]):
        Wg = np.asarray(Wg, np.float32)
        for i in range(2):
            wfeat[i * F:(i + 1) * F, g * U:(g + 1) * U] = Wg[0:F, :] / float(H * W)
        whid[:, g * U:(g + 1) * U] = Wg[F:F + U, :]
    gbias = np.concatenate([np.asarray(b, np.float32) for b in (b1, b2, b3)])
    gbias = gbias.reshape(1, 96)

    return xin2, smat, cbias, wfeat, whid, gbias


# ------------------------------------------------------------------ kernel
def kernel(x, conv_w, conv_b, W1, b1, W2, b2, W3, b3, W4, b4):
    global LAST_RESULTS
    xin2, smat, cbias, wfeat, whid, gbias = _prep_inputs(
        x, conv_w, conv_b, W1, b1, W2, b2, W3, b3)

    nc = _build_bass(use_gbias=bool(np.any(gbias)))
    in_maps = [{
        'xin': np.ascontiguousarray(xin2[b]),
        'smat': smat,
        'cbias': cbias,
        'wfeat': wfeat,
        'whid': whid,
        'gbias': gbias,
    } for b in range(B)]

    res = run_bass_kernel_spmd(nc, in_maps, core_ids=list(range(B)))
    LAST_RESULTS = res
    out = np.stack([res.results[b]['outh'][0] for b in range(B)], axis=0)
    return out.astype(np.float32)


# revision 26
# speedup vs baseline: 1.0633x; 1.0574x over previous
"""Trainium2 Bass kernel for BasicCNN+LSTM (conv3x3+ReLU+GAP -> custom LSTM scan).

Self-contained: hardcodes shapes/sharding. Data-parallel over batch B=8 across
8 NeuronCores; each core processes one batch element end-to-end, the host
gathers the 8 [1,32] results.

Per-core device pipeline (per frame t of 24):
  - DMA a host-prepacked, channel-deinterleaved "stack" [36, 56*112] bf16 into
    an SBUF row-band (partition p = c*12 + dx*4 + r holds plane c shifted by
    (dx-1, parity row r)); 2 round-robin bands at partitions {0, 64} -> 2
    concurrent PE row-group streams.
  - Conv as ONE K=36 matmul per PSUM tile (contraction over the full 3x3x3
    receptive field of a vertically-packed pixel pair; M=96 = 2 px x 48
    filters, N=448 = 4 ja-blocks x 112 w, contiguous rhs). 14 tiles/frame.
  - Fused ReLU(+conv-bias)+GAP: ScalarE activation(Relu, bias, accum_out) and
    VectorE tensor_scalar((x+bias) max 0, accum_out), 7/7 split with separate
    per-engine gsum tiles (avoids cross-engine WAW serialization).
  - Tiny LSTM-ish scan step on-chip ([1,96] gates in free layout; the
    reference's state-order swap bug is reproduced faithfully). Scan step t
    is emitted after frame t+2's conv to avoid FIFO head-of-line blocking.
"""
import sys
if '/opt/trn_rl_repo' not in sys.path:
    sys.path.insert(0, '/opt/trn_rl_repo')

import numpy as np
import ml_dtypes

import concourse.bass as bass
import concourse.mybir as mybir
import concourse.tile as tile
from concourse.vector_clock import ScopedClock
from concourse.bass_utils import run_bass_kernel_spmd

# ---------------------------------------------------------------- constants
B, T, H, W, C, F, U = 8, 24, 112, 112, 3, 48, 32
JA = 56            # vertical pixel-pair blocks (112 rows / 2)
KP = 36            # stack partitions: 3 c x 3 dx x 4 window rows
M = 96             # 2 pixels x 48 filters
NSB = 14           # superblocks (PSUM tiles) per frame
NQ = 448           # columns per superblock = 4 ja-blocks x 112 w
FREE = JA * W      # stack free size per partition (elements)

FP32 = mybir.dt.float32
BF16 = mybir.dt.bfloat16

LAST_RESULTS = None  # BassKernelResults of the most recent run (for test.py)

# ------------------------------------------------- TileContext drain patch
# The container's walrus rejects >1 semaphore wait per instruction; Tile's
# kernel-tail drain aggregates all end-of-kernel waits onto one Drain.
# Spread them across single-wait NOPs on the sync engine instead.
def _patched_drain_and_barrier(self, tick_clock, wait_clock):
    nc = self.nc
    probe = nc.sync.nop(nofuse=True, hint="tail_waits")
    wait_clock.add_sem_waits(probe.ins, ScopedClock({None: tick_clock.global_clock}))
    waits = list(probe.ins.sync_info.on_wait or [])
    if len(waits) > 1:
        probe.ins.sync_info.on_wait = waits[:1]
        for i in range(1, len(waits)):
            extra = nc.sync.nop(nofuse=True, hint=f"tail_waits_{i}")
            si = extra.ins.sync_info
            if si is None:
                extra.ins.sync_info = mybir.SyncInfo(on_wait=[waits[i]], on_update=[])
            else:
                si.on_wait = [waits[i]]
    nc.sync.drain()
    nc.all_engine_barrier()
    popped = nc._tile_sem_poison_stack.pop()
    assert popped is self._sem_poison
    nc.clear_and_free_semaphores(list(self.sems.allocated().values()))
    nc.all_engine_barrier()


tile.TileContext._drain_and_barrier = _patched_drain_and_barrier

# Same walrus restriction for regular instructions: spill extra sem waits
# onto preceding same-engine NOPs at commit time.
_orig_commit = tile.TileContext._commit_instruction


def _patched_commit(self, inst, *args, **kwargs):
    si = getattr(inst, 'sync_info', None)
    if si is not None and si.on_wait and len(si.on_wait) > 1 \
            and inst.engine != mybir.EngineType.Unassigned:
        waits = list(si.on_wait)
        si.on_wait = waits[-1:]
        for w in waits[:-1]:
            nop = mybir.InstNoOp(
                name=self.nc.get_next_instruction_name(),
                ins=[], outs=[], bass_is_fusable=False)
            nop.engine = inst.engine
            nop.sync_info = mybir.SyncInfo(on_wait=[w], on_update=[])
            _orig_commit(self, nop, *args, **kwargs)
    return _orig_commit(self, inst, *args, **kwargs)


tile.TileContext._commit_instruction = _patched_commit

# NOTE: tried --enable-ldw-opt=true to dedupe the per-matmul stationary
# reloads (~70us of PE time); this walrus build fails in visitInstLdweights
# with it enabled, so the flag stays off.


# ------------------------------------------------------------- device code
def _build_bass(use_gbias=True):
    _build_bass.use_gbias = use_gbias
    nc = bass.Bass('TRN2', target_bir_lowering=False, debug=False)

    xin = nc.dram_tensor('xin', [T, KP, FREE], BF16, kind='ExternalInput')
    smat_d = nc.dram_tensor('smat', [KP, M], BF16, kind='ExternalInput')
    cbias_d = nc.dram_tensor('cbias', [M, 1], FP32, kind='ExternalInput')
    wfeat_d = nc.dram_tensor('wfeat', [M, 96], FP32, kind='ExternalInput')
    whid_d = nc.dram_tensor('whid', [U, 96], FP32, kind='ExternalInput')
    gbias_d = nc.dram_tensor('gbias', [1, 96], FP32, kind='ExternalInput')
    outh_d = nc.dram_tensor('outh', [1, U], FP32, kind='ExternalOutput')

    Relu = mybir.ActivationFunctionType.Relu
    Sigmoid = mybir.ActivationFunctionType.Sigmoid
    Tanh = mybir.ActivationFunctionType.Tanh
    Amax = mybir.AluOpType.max
    Aadd = mybir.AluOpType.add

    with tile.TileContext(nc) as tc:
        const = tc.alloc_tile_pool(name='const', bufs=1)
        state = tc.alloc_tile_pool(name='state', bufs=1)
        stackp = tc.alloc_tile_pool(name='stack', bufs=3)
        psum = tc.alloc_tile_pool(name='psum', bufs=3, space='PSUM')
        spsum = tc.alloc_tile_pool(name='spsum', bufs=1, space='PSUM')
        gs = tc.alloc_tile_pool(name='gs', bufs=6)
        fs = tc.alloc_tile_pool(name='fs', bufs=6)
        ga_pool = tc.alloc_tile_pool(name='ga', bufs=4)
        tmp = tc.alloc_tile_pool(name='tmp', bufs=6)

        # constants
        sc_all = const.tile([128, M], BF16, tag='sc')
        for s in range(2):
            nc.sync.dma_start(sc_all[64 * s:64 * s + KP, :], smat_d[:])
        cbias = const.tile([M, 1], FP32, tag='cb')
        nc.sync.dma_start(cbias[:], cbias_d[:])
        wfeat = const.tile([M, 96], FP32, tag='wf')
        nc.sync.dma_start(wfeat[:], wfeat_d[:])
        whid = const.tile([U, 96], FP32, tag='wh')
        nc.sync.dma_start(whid[:], whid_d[:])
        gbias = const.tile([1, 96], FP32, tag='gb')
        nc.sync.dma_start(gbias[:], gbias_d[:])
        ident = const.tile([1, 1], FP32, tag='id')
        nc.vector.memset(ident[:], 1.0)
        zeros1k = const.tile([M, 1024], FP32, tag='z1k')
        nc.vector.memset(zeros1k[:], 0.0)

        # persistent scan state
        cellv = state.tile([1, U], FP32, tag='cell')    # prev new_cell
        hidv = state.tile([1, U], FP32, tag='hid')      # prev new_hidden
        cell_part = state.tile([U, 1], FP32, tag='cp')  # new_cell, transposed
        nc.vector.memset(cellv[:], 0.0)
        nc.vector.memset(hidv[:], 0.0)
        nc.vector.memset(cell_part[:], 0.0)

        fsums = [None] * T

        # Six 1024-col double-bank PSUM windows (2 matmuls, ONE fused
        # relu+accum each: halves the per-instruction + accumulator-read
        # overhead) + one 128-col tail, per frame. ACT 3 / DVE 3+tail.
        rounds = [None] * (T // 2)

        def get_round(g):
            if rounds[g] is None:
                rt = stackp.tile([128, FREE], BF16, tag='stk')
                nc.sync.dma_start(rt[0:KP, :], xin[2 * g])
                nc.sync.dma_start(rt[64:64 + KP, :], xin[2 * g + 1])
                rounds[g] = rt
            return rounds[g]

        def emit_conv(t):
            s = t % 2
            rt = get_round(t // 2)
            if s == 0 and t // 2 + 1 < T // 2:
                get_round(t // 2 + 1)  # prefetch next round's DMAs
            band = rt[64 * s:64 * s + KP, :]
            lhsT = sc_all[64 * s:64 * s + KP, :]

            gsumA = gs.tile([M, 4], FP32, tag='gsumA')
            gsumB = gs.tile([M, 3], FP32, tag='gsumB')
            zv = zeros1k.rearrange("p (b n) -> p b n", b=2)[:, :, 0:448]
            ia = ib = 0
            for k in range(7):
                # two 448-col matmuls at bank-aligned offsets 0/512, then one
                # fused relu+accum over the strided [96, 2, 448] view
                ps = psum.tile([M, 1024], FP32, tag='ps')
                for h in range(2):
                    off = k * 896 + h * 448
                    nc.tensor.matmul(ps[:, h * 512:h * 512 + 448], lhsT,
                                     band[:, off:off + 448],
                                     start=True, stop=True,
                                     tile_position=(64 * s, 0))
                psv = ps.rearrange("p (b n) -> p b n", b=2)[:, :, 0:448]
                if k % 2 == 0:
                    nc.scalar.activation(psv, psv, Relu, bias=cbias[:],
                                         accum_out=gsumA[:, ia:ia + 1])
                    ia += 1
                else:
                    nc.vector.scalar_tensor_tensor(
                        out=psv, in0=psv, scalar=cbias[:],
                        in1=zv, op0=Aadd, op1=Amax,
                        accum_out=gsumB[:, ib:ib + 1])
                    ib += 1

            fsA = tmp.tile([M, 1], FP32, tag='fsA')
            nc.vector.reduce_sum(fsA[:], gsumA[:], axis=mybir.AxisListType.X)
            fsB = tmp.tile([M, 1], FP32, tag='fsB')
            nc.vector.reduce_sum(fsB[:], gsumB[:], axis=mybir.AxisListType.X)
            fsum = fs.tile([M, 1], FP32, tag='fsum')
            nc.vector.tensor_add(fsum[:], fsA[:], fsB[:])
            fsums[t] = fsum

        def emit_scan(t):
            # z-hidden part = prev new_cell (reference's state-order swap bug)
            fsum = fsums[t]
            pg = spsum.tile([1, 96], FP32, tag='sps')
            nc.tensor.matmul(pg[:], fsum[:], wfeat[:], start=True, stop=False)
            nc.tensor.matmul(pg[:], cell_part[:], whid[:], start=False, stop=True)
            if _build_bass.use_gbias:
                gpre = ga_pool.tile([1, 96], FP32, tag='gpre')
                nc.vector.tensor_add(gpre[:], pg[:], gbias[:])
            else:
                gpre = pg
            ga = ga_pool.tile([1, 96], FP32, tag='ga')
            nc.scalar.activation(ga[:, 0:2 * U], gpre[:, 0:2 * U], Sigmoid)
            nc.scalar.activation(ga[:, 2 * U:3 * U], gpre[:, 2 * U:3 * U], Tanh)
            t1 = tmp.tile([1, U], FP32, tag='t1')
            nc.vector.tensor_mul(t1[:], ga[:, 0:U], hidv[:])       # sig1*prev_hid
            t2 = tmp.tile([1, U], FP32, tag='t2')
            nc.vector.tensor_mul(t2[:], ga[:, U:2 * U], ga[:, 2 * U:3 * U])
            nc.vector.tensor_add(cellv[:], t1[:], t2[:])           # new_cell
            t3 = tmp.tile([1, U], FP32, tag='t3')
            nc.scalar.activation(t3[:], cellv[:], Tanh)
            nc.vector.tensor_mul(hidv[:], cellv[:], t3[:])         # new_hidden
            if t < T - 1:
                ph = spsum.tile([U, 1], FP32, tag='sps')
                nc.tensor.transpose(ph[:], cellv[:], ident[:])
                nc.vector.tensor_copy(cell_part[:], ph[:])

        LAG = 4  # scan step t emitted alongside frame t+LAG's conv: its
        # deps are LAG frames old, so it never head-of-line blocks a queue.
        for t in range(T):
            if t >= LAG:
                emit_scan(t - LAG)
            emit_conv(t)
        for t in range(T - LAG, T):
            emit_scan(t)

        nc.sync.dma_start(outh_d[:], hidv[:])

        for p in (tmp, ga_pool, fs, gs, spsum, psum, stackp, state, const):
            p.release()

    return nc


# -------------------------------------------------------------- host prep
def _prep_inputs(x, conv_w, conv_b, W1, b1, W2, b2, W3, b3):
    x = np.asarray(x, np.float32)
    conv_w = np.asarray(conv_w, np.float32)
    conv_b = np.asarray(conv_b, np.float32)

    xp = np.zeros((B, T, H + 2, W + 2, C), np.float32)
    xp[:, :, 1:H + 1, 1:W + 1, :] = x
    xin2 = np.empty((B, T, KP, JA, W), np.float32)
    rows = 2 * np.arange(JA)
    for c in range(3):
        for dx in range(3):
            for r in range(4):
                p = c * 12 + dx * 4 + r
                xin2[:, :, p] = np.moveaxis(
                    xp[:, :, rows + r, dx:dx + W, c], 0, 2)
    xin2 = xin2.reshape(B, T, KP, FREE).astype(ml_dtypes.bfloat16)

    smat = np.zeros((KP, M), np.float32)
    for c in range(3):
        for dx in range(3):
            for r in range(4):
                p = c * 12 + dx * 4 + r
                for i in range(2):
                    dy = r - i
                    if 0 <= dy <= 2:
                        smat[p, i * F:(i + 1) * F] = conv_w[dy, dx, c, :]
    smat = smat.astype(ml_dtypes.bfloat16)
    cbias = np.concatenate([conv_b, conv_b]).reshape(M, 1).astype(np.float32)

    wfeat = np.zeros((M, 96), np.float32)
    whid = np.zeros((U, 96), np.float32)
    for g, Wg in enumerate([W1, W2, W3]):
        Wg = np.asarray(Wg, np.float32)
        for i in range(2):
            wfeat[i * F:(i + 1) * F, g * U:(g + 1) * U] = Wg[0:F, :] / float(H * W)
        whid[:, g * U:(g + 1) * U] = Wg[F:F + U, :]
    gbias = np.concatenate([np.asarray(b, np.float32) for b in (b1, b2, b3)])
    gbias = gbias.reshape(1, 96)

    return xin2, smat, cbias, wfeat, whid, gbias


# ------------------------------------------------------------------ kernel
def kernel(x, conv_w, conv_b, W1, b1, W2, b2, W3, b3, W4, b4):
    global LAST_RESULTS
    xin2, smat, cbias, wfeat, whid, gbias = _prep_inputs(
        x, conv_w, conv_b, W1, b1, W2, b2, W3, b3)

    nc = _build_bass(use_gbias=bool(np.any(gbias)))
    in_maps = [{
        'xin': np.ascontiguousarray(xin2[b]),
        'smat': smat,
        'cbias': cbias,
        'wfeat': wfeat,
        'whid': whid,
        'gbias': gbias,
    } for b in range(B)]

    res = run_bass_kernel_spmd(nc, in_maps, core_ids=list(range(B)))
    LAST_RESULTS = res
    out = np.stack([res.results[b]['outh'][0] for b in range(B)], axis=0)
    return out.astype(np.float32)
